# revision 8
# baseline (speedup 1.0000x reference)
"""Trainium2 Bass kernel for MetaPartModule (slot-attention style, 3 iterations).

Sharding: data-parallel over batch b (64) across 8 cores (8 batches/core).
BatchNorm statistics are made exact-global via two small AllReduces per
iteration (per-feature q stats, per-channel attn stats).

Per-core layout:
  - local batches grouped as 2 groups x 4 batches; packed partition row
    p = 32*bhat + i  (bhat = batch-in-group, i = slot index), bb = 4*g + bhat.
  - x needed in two majors: d-on-partitions ("xn", for attn = q @ k^T) and
    hw-on-partitions ("xt", for update = attn_n @ v). Both bf16; part
    resident in SBUF, rest streamed from HBM each iteration.
"""

import numpy as np
import ml_dtypes

import concourse.bass as bass
import concourse.tile as tile
from concourse import bacc, mybir
from concourse.bass_utils import run_bass_kernel_spmd
from concourse.masks import make_identity

F32 = mybir.dt.float32
BF16 = mybir.dt.bfloat16
AF = mybir.ActivationFunctionType
ALU = mybir.AluOpType
AX = mybir.AxisListType

N_CORES = 8
B = 64
C = 512
HW = 4096
N = 32
B_LOC = B // N_CORES      # 8
G = 2                     # batch groups per core
BH = 4                    # batches per group
DT = 4                    # d tiles of 128
JT = 32                   # hw tiles of 128
NT = 8                    # hw chunks of 512
ITERS = 3
ALPHA = 0.1
BN_EPS = 1e-5
EPS = 1e-12

NT_RES = 2                # resident hw chunks (of 512) for xn, per batch
JT_RES = 5                # resident hw tiles (of 128) for xt, per batch

_CACHE = {}


def _build():
    if "nc" in _CACHE:
        return _CACHE["nc"]
    nc = bacc.Bacc("TRN2", target_bir_lowering=False, debug=False,
                   num_devices=N_CORES)

    xn_h = nc.dram_tensor("xn", [B_LOC, C, HW], BF16, kind="ExternalInput")
    xt_h = nc.dram_tensor("xt", [B_LOC, JT, 128, C], BF16, kind="ExternalInput")
    wq_h = nc.dram_tensor("wq", [128, DT, C], BF16, kind="ExternalInput")
    mT_h = nc.dram_tensor("mT", [128, DT, N], BF16, kind="ExternalInput")
    m128_h = nc.dram_tensor("m128", [128, C], F32, kind="ExternalInput")
    g1b1_h = nc.dram_tensor("g1b1", [128, DT, 2], F32, kind="ExternalInput")
    g2b2_h = nc.dram_tensor("g2b2", [128, 2], F32, kind="ExternalInput")
    fold_h = nc.dram_tensor("fold", [128, 128], F32, kind="ExternalInput")
    out_h = nc.dram_tensor("out", [G, 128, C], F32, kind="ExternalOutput")

    with tile.TileContext(nc) as tc:
        with (
            tc.tile_pool(name="const", bufs=1) as constp,
            tc.tile_pool(name="xres", bufs=1) as xres,
            tc.tile_pool(name="attn", bufs=1) as attnp,
            tc.tile_pool(name="state", bufs=1) as statep,
            tc.tile_pool(name="stats", bufs=2) as statsp,
            tc.tile_pool(name="small", bufs=2) as smallp,
            tc.tile_pool(name="upd", bufs=1) as updp,
            tc.tile_pool(name="xn_ring", bufs=6) as xnring,
            tc.tile_pool(name="xt_ring", bufs=4) as xtring,
            tc.tile_pool(name="dram", bufs=2, space="DRAM") as dram,
            tc.tile_pool(name="pa", bufs=2, space="PSUM") as pa,
            tc.tile_pool(name="pu", bufs=2, space="PSUM") as pu,
            tc.tile_pool(name="pq", bufs=2, space="PSUM") as pq,
            tc.tile_pool(name="ptr", bufs=1, space="PSUM") as ptr,
        ):
            # ---- constants ----
            wq_sb = constp.tile([128, DT, C], BF16)
            nc.sync.dma_start(wq_sb[:], wq_h[:])
            mT_sb = constp.tile([128, DT, N], BF16)
            nc.sync.dma_start(mT_sb[:], mT_h[:])
            g1b1_sb = constp.tile([128, DT, 2], F32)
            nc.sync.dma_start(g1b1_sb[:], g1b1_h[:])
            g2b2_sb = constp.tile([128, 2], F32)
            nc.sync.dma_start(g2b2_sb[:], g2b2_h[:])
            fold_sb = constp.tile([128, 128], F32)
            nc.sync.dma_start(fold_sb[:], fold_h[:])
            ident = constp.tile([128, 128], F32)
            make_identity(nc, ident[:])
            eps_sb = constp.tile([128, 1], F32)
            nc.gpsimd.memset(eps_sb[:], BN_EPS)

            # ---- state ----
            slots_sb = statep.tile([128, G, C], F32)
            for g in range(G):
                nc.sync.dma_start(slots_sb[:, g, :], m128_h[:])
            slotsT_sb = statep.tile([128, DT, G * 128], BF16)
            q_sb = statep.tile([128, DT, G * 128], F32)
            qbn_sb = statep.tile([128, DT, G * 128], BF16)
            out_stage = statep.tile([128, G, C], F32)

            # ---- attn working set ----
            attn_raw = attnp.tile([128, G, HW], BF16)
            attn_nT = attnp.tile([128, G, JT, 128], BF16)
            rs_sb = attnp.tile([128, G], F32)
            recip_sb = attnp.tile([128, G], F32)

            # ---- resident x ----
            xn_res = xres.tile([128, B_LOC, DT, NT_RES * 512], BF16)
            for bb in range(B_LOC):
                nc.sync.dma_start(
                    xn_res[:, bb, :, :],
                    xn_h[bb, :, 0:NT_RES * 512].rearrange(
                        "(dt p) w -> p dt w", p=128),
                )
            xt_res = xres.tile([128, B_LOC, JT_RES, C], BF16)
            for bb in range(B_LOC):
                nc.sync.dma_start(
                    xt_res[:, bb, :, :],
                    xt_h[bb, 0:JT_RES].rearrange("jt p d -> p jt d"),
                )

            for t in range(ITERS):
                # ============ Q phase ============
                if t == 0:
                    # slots identical across batches: q over the 32 unique rows
                    nrow = N
                    q_rhs = lambda dt_i: mT_sb[:, dt_i, :]
                else:
                    nrow = G * 128
                    # transpose slots (row-major packed) -> slotsT (d-major)
                    for g in range(G):
                        for dc in range(DT):
                            ps_tr = ptr.tile([128, 128], F32)
                            nc.tensor.transpose(
                                ps_tr[:], slots_sb[:, g, dc * 128:(dc + 1) * 128],
                                ident[:])
                            nc.vector.tensor_copy(
                                slotsT_sb[:, dc, g * 128:(g + 1) * 128], ps_tr[:])
                    q_rhs = lambda dt_i: slotsT_sb[:, dt_i, :]

                qmv = statsp.tile([128, DT, 2], F32, tag="qmv")
                for dt_o in range(DT):
                    ps_q = pq.tile([128, G * 128], F32, tag="psq")
                    for dt_i in range(DT):
                        nc.tensor.matmul(
                            ps_q[:, 0:nrow],
                            wq_sb[:, dt_i, dt_o * 128:(dt_o + 1) * 128],
                            q_rhs(dt_i),
                            start=(dt_i == 0), stop=(dt_i == DT - 1),
                        )
                    nc.vector.tensor_copy(q_sb[:, dt_o, 0:nrow], ps_q[:, 0:nrow])
                    bnst = statsp.tile([128, 6], F32, tag="bnst")
                    nc.vector.bn_stats(bnst[:], q_sb[:, dt_o, 0:nrow])
                    nc.vector.bn_aggr(qmv[:, dt_o, :], bnst[:])

                a1_sb = statsp.tile([128, DT], F32, tag="a1")
                c1_sb = statsp.tile([128, DT], F32, tag="c1")
                tmp4 = statsp.tile([128, DT], F32, tag="tmp4")
                sd4 = statsp.tile([128, DT], F32, tag="sd4")
                if t == 0:
                    mu_ap = qmv[:, :, 0]
                    var_ap = qmv[:, :, 1]
                    nc.scalar.activation(sd4[:], var_ap, AF.Sqrt, bias=eps_sb[:])
                    nc.vector.reciprocal(a1_sb[:], sd4[:])
                    nc.vector.tensor_mul(a1_sb[:], a1_sb[:], g1b1_sb[:, :, 0])
                    nc.vector.tensor_mul(tmp4[:], mu_ap, a1_sb[:])
                    nc.vector.tensor_sub(c1_sb[:], g1b1_sb[:, :, 1], tmp4[:])
                else:
                    # local (mean, E2) -> AllReduce -> global stats
                    arq = statsp.tile([128, DT, 2], F32, tag="arq")
                    nc.vector.tensor_copy(arq[:, :, 0], qmv[:, :, 0])
                    nc.vector.tensor_mul(tmp4[:], qmv[:, :, 0], qmv[:, :, 0])
                    nc.vector.tensor_add(arq[:, :, 1], qmv[:, :, 1], tmp4[:])
                    qcc_in = dram.tile([128, DT, 2], F32, tag="qcc_in")
                    qcc_out = dram.tile([128, DT, 2], F32, tag="qcc_out")
                    nc.gpsimd.dma_start(qcc_in[:], arq[:])
                    nc.gpsimd.collective_compute(
                        "AllReduce", ALU.add,
                        replica_groups=[list(range(N_CORES))],
                        ins=[qcc_in.opt()], outs=[qcc_out.opt()],
                    )
                    gq = statsp.tile([128, DT, 2], F32, tag="gq")
                    nc.gpsimd.dma_start(gq[:], qcc_out[:])
                    mu4 = statsp.tile([128, DT], F32, tag="mu4")
                    nc.vector.tensor_scalar_mul(mu4[:], gq[:, :, 0], 1.0 / N_CORES)
                    e24 = statsp.tile([128, DT], F32, tag="e24")
                    nc.vector.tensor_scalar_mul(e24[:], gq[:, :, 1], 1.0 / N_CORES)
                    nc.vector.tensor_mul(tmp4[:], mu4[:], mu4[:])
                    nc.vector.tensor_sub(e24[:], e24[:], tmp4[:])  # var
                    nc.scalar.activation(sd4[:], e24[:], AF.Sqrt, bias=eps_sb[:])
                    nc.vector.reciprocal(a1_sb[:], sd4[:])
                    nc.vector.tensor_mul(a1_sb[:], a1_sb[:], g1b1_sb[:, :, 0])
                    nc.vector.tensor_mul(tmp4[:], mu4[:], a1_sb[:])
                    nc.vector.tensor_sub(c1_sb[:], g1b1_sb[:, :, 1], tmp4[:])

                for dt_o in range(DT):
                    nc.scalar.activation(
                        qbn_sb[:, dt_o, 0:nrow], q_sb[:, dt_o, 0:nrow], AF.Relu,
                        scale=a1_sb[:, dt_o:dt_o + 1], bias=c1_sb[:, dt_o:dt_o + 1])

                # ============ ATTN phase ============
                bnsta = statsp.tile([128, G * NT, 6], F32, tag="bnsta")
                for g in range(G):
                    for nt in range(NT):
                        if nt < NT_RES:
                            rhs = lambda bh, dt: xn_res[
                                :, 4 * g + bh, dt, nt * 512:(nt + 1) * 512]
                        else:
                            rts = []
                            for bh in range(BH):
                                rt = xnring.tile([128, DT, 512], BF16, tag="xnr")
                                nc.sync.dma_start(
                                    rt[:],
                                    xn_h[4 * g + bh, :, nt * 512:(nt + 1) * 512]
                                    .rearrange("(dt p) w -> p dt w", p=128),
                                )
                                rts.append(rt)
                            rhs = lambda bh, dt, rts=rts: rts[bh][:, dt, :]
                        ps_a = pa.tile([128, 512], F32, tag="psa")
                        for dt in range(DT):
                            for bh in range(BH):
                                if t == 0:
                                    lhsT = qbn_sb[:, dt, 0:N]
                                else:
                                    lhsT = qbn_sb[:, dt,
                                                  g * 128 + 32 * bh:
                                                  g * 128 + 32 * bh + 32]
                                nc.tensor.matmul(
                                    ps_a[32 * bh:32 * bh + 32, :],
                                    lhsT, rhs(bh, dt),
                                    start=(dt == 0), stop=(dt == DT - 1),
                                    tile_position=(0, 32 * bh),
                                )
                        nc.vector.bn_stats(bnsta[:, g * NT + nt, :], ps_a[:])
                        nc.vector.tensor_copy(
                            attn_raw[:, g, nt * 512:(nt + 1) * 512], ps_a[:])

                amv = statsp.tile([128, 2], F32, tag="amv")
                nc.vector.bn_aggr(amv[:], bnsta[:])
                ar2 = statsp.tile([128, 2], F32, tag="ar2")
                tmp1 = statsp.tile([128, 1], F32, tag="tmp1")
                nc.vector.tensor_copy(ar2[:, 0:1], amv[:, 0:1])
                nc.vector.tensor_mul(tmp1[:], amv[:, 0:1], amv[:, 0:1])
                nc.vector.tensor_add(ar2[:, 1:2], amv[:, 1:2], tmp1[:])
                acc_in = dram.tile([128, 2], F32, tag="acc_in")
                acc_out = dram.tile([128, 2], F32, tag="acc_out")
                nc.gpsimd.dma_start(acc_in[:], ar2[:])
                nc.gpsimd.collective_compute(
                    "AllReduce", ALU.add,
                    replica_groups=[list(range(N_CORES))],
                    ins=[acc_in.opt()], outs=[acc_out.opt()],
                )
                ga = statsp.tile([128, 2], F32, tag="ga")
                nc.gpsimd.dma_start(ga[:], acc_out[:])

                # fold stats across the 4 bhat blocks (and broadcast to all
                # partitions of the same channel) with one masked matmul
                ps_f = ptr.tile([128, 2], F32, tag="psf")
                nc.tensor.matmul(ps_f[:], fold_sb[:], ga[:])
                mu1 = smallp.tile([128, 1], F32, tag="mu1")
                nc.vector.tensor_scalar_mul(mu1[:], ps_f[:, 0:1], 1.0 / (4 * N_CORES))
                e21 = smallp.tile([128, 1], F32, tag="e21")
                nc.vector.tensor_scalar_mul(e21[:], ps_f[:, 1:2], 1.0 / (4 * N_CORES))
                var1 = smallp.tile([128, 1], F32, tag="var1")
                nc.vector.tensor_mul(var1[:], mu1[:], mu1[:])
                nc.vector.tensor_sub(var1[:], e21[:], var1[:])
                sd1 = smallp.tile([128, 1], F32, tag="sd1")
                nc.scalar.activation(sd1[:], var1[:], AF.Sqrt, bias=eps_sb[:])
                ac128 = smallp.tile([128, 2], F32, tag="ac128")
                nc.vector.reciprocal(ac128[:, 0:1], sd1[:])
                nc.vector.tensor_mul(ac128[:, 0:1], ac128[:, 0:1], g2b2_sb[:, 0:1])
                nc.vector.tensor_mul(ac128[:, 1:2], mu1[:], ac128[:, 0:1])
                nc.vector.tensor_sub(ac128[:, 1:2], g2b2_sb[:, 1:2], ac128[:, 1:2])

                # normalize + relu + rowsum, then transpose attn_n
                for g in range(G):
                    nc.scalar.activation(
                        attn_raw[:, g, :], attn_raw[:, g, :], AF.Relu,
                        scale=ac128[:, 0:1], bias=ac128[:, 1:2],
                        accum_out=rs_sb[:, g:g + 1])
                    nc.sync.dma_start_transpose(
                        attn_nT[:, g, :, :], attn_raw[:, g, :])
                nc.vector.tensor_scalar_add(recip_sb[:], rs_sb[:], EPS)
                nc.vector.reciprocal(recip_sb[:], recip_sb[:])

                # ============ UPDATE phase ============
                for g in range(G):
                    ps_u = pu.tile([128, C], F32, tag="psu")
                    for jt in range(JT):
                        if jt < JT_RES:
                            rhs_u = lambda bh, jt=jt: xt_res[:, 4 * g + bh, jt, :]
                        else:
                            ut = xtring.tile([128, BH, C], BF16, tag="xtr")
                            nc.sync.dma_start(
                                ut[:],
                                xt_h[4 * g:4 * g + 4, jt].rearrange(
                                    "b p d -> p b d"),
                            )
                            rhs_u = lambda bh, ut=ut: ut[:, bh, :]
                        for bh in range(BH):
                            nc.tensor.matmul(
                                ps_u[32 * bh:32 * bh + 32, :],
                                attn_nT[:, g, jt, 32 * bh:32 * bh + 32],
                                rhs_u(bh),
                                start=(jt == 0), stop=(jt == JT - 1),
                                tile_position=(0, 32 * bh),
                            )
                    if t < ITERS - 1:
                        tu = updp.tile([128, C], F32, tag="tu")
                        nc.vector.tensor_scalar(
                            tu[:], ps_u[:], recip_sb[:, g:g + 1], ALPHA,
                            op0=ALU.mult, op1=ALU.mult)
                        ts_ = updp.tile([128, C], F32, tag="ts")
                        nc.vector.tensor_scalar_mul(
                            ts_[:], slots_sb[:, g, :], 1.0 - ALPHA)
                        nc.vector.tensor_add(slots_sb[:, g, :], tu[:], ts_[:])
                    else:
                        nc.vector.tensor_scalar(
                            out_stage[:, g, :], ps_u[:], recip_sb[:, g:g + 1],
                            None, op0=ALU.mult)

            for g in range(G):
                nc.sync.dma_start(out_h[g], out_stage[:, g, :])

    nc.compile()
    _CACHE["nc"] = nc
    return nc


def _prep_inputs(x, meta_embed, Wq, g1, b1, g2, b2):
    bf16 = ml_dtypes.bfloat16
    x3 = np.asarray(x, dtype=np.float32).reshape(B, C, HW)
    Wq = np.asarray(Wq, dtype=np.float32)
    meta = np.asarray(meta_embed, dtype=np.float32)
    wq_t = np.ascontiguousarray(
        Wq.T.reshape(DT, 128, C).transpose(1, 0, 2)).astype(bf16)
    mT_t = np.ascontiguousarray(
        meta.T.reshape(DT, 128, N).transpose(1, 0, 2)).astype(bf16)
    m128 = np.ascontiguousarray(np.tile(meta, (BH, 1))).astype(np.float32)
    g1b1 = np.stack([np.asarray(g1, np.float32).reshape(DT, 128).T,
                     np.asarray(b1, np.float32).reshape(DT, 128).T], axis=-1)
    g1b1 = np.ascontiguousarray(g1b1)
    g2b2 = np.stack([np.tile(np.asarray(g2, np.float32), BH),
                     np.tile(np.asarray(b2, np.float32), BH)], axis=-1)
    g2b2 = np.ascontiguousarray(g2b2)
    fold = np.tile(np.eye(N, dtype=np.float32), (BH, BH))
    fold = np.ascontiguousarray(fold)

    in_maps = []
    for c in range(N_CORES):
        sl = x3[c * B_LOC:(c + 1) * B_LOC]
        xn = np.ascontiguousarray(sl).astype(bf16)
        xt = np.ascontiguousarray(sl.transpose(0, 2, 1)).reshape(
            B_LOC, JT, 128, C).astype(bf16)
        in_maps.append({
            "xn": xn, "xt": xt, "wq": wq_t, "mT": mT_t, "m128": m128,
            "g1b1": g1b1, "g2b2": g2b2, "fold": fold,
        })
    return in_maps


def _unpack(results):
    out = np.empty((B, N, C), dtype=np.float32)
    for c in range(N_CORES):
        r = results[c]["out"]          # [G, 128, C]
        for g in range(G):
            blk = r[g].reshape(BH, N, C)
            for bh in range(BH):
                out[c * B_LOC + g * BH + bh] = blk[bh]
    return out


def run(trace=False, **inputs):
    nc = _build()
    in_maps = _prep_inputs(
        inputs["x"], inputs["meta_embed"], inputs["Wq"],
        inputs["g1"], inputs["b1"], inputs["g2"], inputs["b2"])
    res = run_bass_kernel_spmd(nc, in_maps, core_ids=list(range(N_CORES)),
                               trace=trace)
    return _unpack(res.results), res


def kernel(**inputs):
    out, _ = run(trace=False, **inputs)
    return out
